# revision 31
# baseline (speedup 1.0000x reference)
"""Causal multi-head attention (B=2, T=2048, C=2048, H=16) on 8 TRN2 NeuronCores.

Sharding (v5): 2D batch x head-group. Cores are split into 2 groups of 4;
group g owns batch g. Within a group each core owns 4 heads (columns of
Wq/Wk/Wv, rows of Wo). Each core reads only its batch's activations
(8MB instead of 16MB) and writes an [2048, 2048] f16 partial (8MB instead
of 16MB); the host sums the 4 partials per batch and adds bo. Halving
per-core DRAM traffic attacks the 8-core DMA/HBM contention that dominated
the head-only sharding.

Kernel structure (per core):
  phase A: per 512-token tile, two head-pair passes compute q/k (transposed
    [d, tok] layout) and v (natural [tok, d]) with PSUM-chain matmuls; the
    next tile's 2MB xT DMA is issued inside the second pass's v-section,
    whose SBUF traffic is light, so the DMA write doesn't contend with the
    q/k streams.
  phase B: per 128-query row and head-pair, K-stationary S^T=[k,q] blocks in
    quartets ([128,512] PSUM), exp on Act one quartet ahead of PV, rowsums
    via ones-matmuls into the same PSUM bank as O^T, reciprocal broadcast
    via gpsimd.
  phase C: out-proj row chunks (4 Wo column chunks x 2 head-halves)
    interleaved between the next row's quartets to spread their SBUF-heavy
    wo streams; per-row [128, 2048] f16 output DMA.
"""

import math
from contextlib import ExitStack

import numpy as np

import concourse.bass as bass
import concourse.tile as tile
from concourse import bacc, mybir
from concourse import bass_utils

F16 = mybir.dt.float16
F32 = mybir.dt.float32
AF = mybir.ActivationFunctionType

B, T, C, H, D = 2, 2048, 2048, 16, 128
NCORES = 8
HPC = 4                      # heads per core
HPAIRS = 2                   # head pairs per core
HD = HPC * D                 # 512 head-cols per core
NTOK = T                     # 2048 tokens per core (one batch)
CCH = C // 128               # 16 contraction chunks
TT = 512                     # projection token tile
NTT = NTOK // TT             # 4
GPB = T // 128               # q-rows = 16
NG = GPB                     # 16 token blocks of 128
SCL = 1.0 / math.sqrt(D)
NEG = -1e30


def _emit(tc: tile.TileContext, reps: int):
    nc = tc.nc
    xT = nc.dram_tensor("xT", [C, NTOK], F16, kind="ExternalInput").ap()
    wq = nc.dram_tensor("wq", [C, HD], F16, kind="ExternalInput").ap()
    wk = nc.dram_tensor("wk", [C, HD], F16, kind="ExternalInput").ap()
    wv = nc.dram_tensor("wv", [C, HD], F16, kind="ExternalInput").ap()
    wo = nc.dram_tensor("wo", [HD, C], F16, kind="ExternalInput").ap()
    bqk = nc.dram_tensor("bqk", [128, 2 * HPC], F32, kind="ExternalInput").ap()
    bv2 = nc.dram_tensor("bv2", [1, HD], F16, kind="ExternalInput").ap()
    out = nc.dram_tensor("out", [NTOK, C], F16, kind="ExternalOutput").ap()

    with ExitStack() as ctx:
        const = ctx.enter_context(tc.tile_pool(name="const", bufs=1))
        persist = ctx.enter_context(tc.tile_pool(name="persist", bufs=1))

        # multiplicative causal mask for P^T diagonal blocks: 1 where
        # k_local <= q_local, 0 above (partition = k, free = q). Applied to
        # PT AFTER the exp (f16 2x-mode DVE mul) instead of an additive f32
        # mask on S^T before it — keeps the DVE off the S->exp critical path.
        tri01 = const.tile([128, 128], F16, tag="tri01")
        nc.gpsimd.memset(tri01, 1.0)
        nc.gpsimd.affine_select(
            out=tri01, in_=tri01, compare_op=mybir.AluOpType.is_ge,
            fill=0.0, base=0, pattern=[[1, 128]], channel_multiplier=-1,
        )
        # all-ones [128,128] rowsum lhsT: out[p,q] = sum_k PT[k,q] for every
        # p, i.e. the rowsum arrives pre-broadcast across partitions. M=128
        # keeps the PE in its fast shape (M=1 outputs measured ~2.7x slower
        # interleaved with PV) and kills the Pool broadcast hop in the tail.
        onesmat = const.tile([128, 128], F16, tag="onesmat")
        nc.vector.memset(onesmat, 1.0)
        onesrow = const.tile([1, 128], F16, tag="onesrow")  # bias lhsT
        nc.vector.memset(onesrow, 1.0)

        # weights: one DMA each, reshaped [C,HD] -> [128, CCH, HD]
        w_sb = {}
        for name, w in (("wq", wq), ("wk", wk), ("wv", wv)):
            t = const.tile([128, CCH, HD], F16, tag=name)
            nc.sync.dma_start(
                t, bass.AP(tensor=w.tensor, offset=w.offset,
                           ap=[[HD, 128], [128 * HD, CCH], [1, HD]]))
            w_sb[name] = t
        wo_sb = const.tile([128, HPC, C], F16, tag="wo")
        nc.sync.dma_start(
            wo_sb, bass.AP(tensor=wo.tensor, offset=wo.offset,
                           ap=[[C, 128], [128 * C, HPC], [1, C]]))

        # biases: bqk [128, 8] = (bq h0..h3, bk h0..h3); bv2 [1, 512]
        bqk_sb = const.tile([128, 2 * HPC], F32, tag="bqk")
        nc.sync.dma_start(bqk_sb, bqk)
        bv2_sb = const.tile([1, HD], F16, tag="bv2")
        nc.sync.dma_start(bv2_sb, bv2)

        qT = persist.tile([128, HPC, NTOK], F16, tag="qT")
        kT = persist.tile([128, HPC, NTOK], F16, tag="kT")
        vN = persist.tile([128, HPC, NG, D], F16, tag="vN")
        OT = persist.tile([128, HPC, NG, 128], F16, tag="OT")

        def body():
            # ---------------- phase A: projections ----------------
            with tc.tile_pool(name="xtp", bufs=2) as xtp, \
                 tc.tile_pool(name="pA", bufs=1, space="PSUM") as pA, \
                 tc.tile_pool(name="pAv", bufs=1, space="PSUM") as pAv:
                xts = {}

                def load_xt(ti):
                    xt = xtp.tile([128, CCH, TT], F16, tag="xt")
                    nc.sync.dma_start(
                        xt, bass.AP(tensor=xT.tensor,
                                    offset=xT.offset + ti * TT,
                                    ap=[[NTOK, 128], [128 * NTOK, CCH],
                                        [1, TT]]))
                    xts[ti] = xt

                load_xt(0)
                load_xt(1)
                for ti in range(NTT):
                    xt = xts.pop(ti)
                    for hp in range(HPAIRS):
                        accs = {}
                        for nm in ("q", "k"):
                            for h2 in range(2):
                                accs[nm, h2] = pA.tile(
                                    [128, TT], F32, tag=f"acc{nm}{h2}",
                                    name=f"acc{nm}{h2}")
                        for c in range(CCH):
                            for h2 in range(2):
                                hh = hp * 2 + h2
                                nc.tensor.matmul(
                                    accs["q", h2],
                                    lhsT=w_sb["wq"][:, c, hh * D:(hh + 1) * D],
                                    rhs=xt[:, c, :], start=(c == 0),
                                    stop=(c == CCH - 1))
                                nc.tensor.matmul(
                                    accs["k", h2],
                                    lhsT=w_sb["wk"][:, c, hh * D:(hh + 1) * D],
                                    rhs=xt[:, c, :], start=(c == 0),
                                    stop=(c == CCH - 1))
                        # xt DMA for ti+2 goes out at the start of the LAST
                        # pass's v-section: v streams ~half the SBUF
                        # bytes/cycle of q/k, so the 2MB DMA write lands in
                        # the SBUF-light window, and the ti buffer's reads
                        # are complete by then (bufs=2 WAR).
                        if hp == HPAIRS - 1 and ti + 2 < NTT:
                            load_xt(ti + 2)
                        # v for this head pair: 256 wv columns
                        vacc = [pAv.tile([128, 512], F32, tag=f"vacc{j}",
                                         name=f"vacc{j}") for j in range(2)]
                        vcol = hp * 256
                        for s in range(4):
                            j, sh = s // 2, s % 2
                            reg = vacc[j][:, sh * 256:(sh + 1) * 256]
                            for c in range(CCH):
                                nc.tensor.matmul(
                                    reg, lhsT=xt[:, c, s * 128:(s + 1) * 128],
                                    rhs=w_sb["wv"][:, c, vcol:vcol + 256],
                                    start=(sh == 0 and c == 0), stop=False)
                            nc.tensor.matmul(
                                reg, lhsT=onesrow,
                                rhs=bv2_sb[:, vcol:vcol + 256],
                                start=False, stop=(sh == 1))
                        # drains spread across engines
                        h0, h1 = hp * 2, hp * 2 + 1
                        nc.scalar.activation(
                            qT[:, h0, ti * TT:(ti + 1) * TT], accs["q", 0],
                            AF.Identity, bias=bqk_sb[:, h0:h0 + 1])
                        nc.vector.tensor_scalar_add(
                            qT[:, h1, ti * TT:(ti + 1) * TT], accs["q", 1],
                            bqk_sb[:, h1:h1 + 1])
                        nc.vector.tensor_scalar_add(
                            kT[:, h0, ti * TT:(ti + 1) * TT], accs["k", 0],
                            bqk_sb[:, HPC + h0:HPC + h0 + 1])
                        nc.scalar.activation(
                            kT[:, h1, ti * TT:(ti + 1) * TT], accs["k", 1],
                            AF.Identity, bias=bqk_sb[:, HPC + h1:HPC + h1 + 1])
                        for j in range(2):
                            # vacc[j] free layout (s, h2, d); dest (h, g, d)
                            g0 = ti * 4 + 2 * j
                            nc.scalar.activation(
                                vN[:, hp * 2:hp * 2 + 2, g0:g0 + 2, :],
                                vacc[j].rearrange("p (s h d) -> p h s d",
                                                  s=2, h=2),
                                AF.Identity)

            # ------------- phase B: attention + phase C: out-proj -------------
            with tc.tile_pool(name="pST", bufs=2, space="PSUM") as pST, \
                 tc.tile_pool(name="ptp", bufs=3) as ptp, \
                 tc.tile_pool(name="pO", bufs=2, space="PSUM") as pO, \
                 tc.tile_pool(name="pC", bufs=1, space="PSUM") as pC, \
                 tc.tile_pool(name="obp", bufs=3) as obp:

                # phase C for row g: 8 chunks (4 wo column chunks x 2
                # head-halves), interleaved between the next row's quartets.
                obs = {}
                pos = {}

                def phase_c_chunk(g, ch):
                    j, half = ch // 2, ch % 2
                    if ch == 0:
                        obs[g] = obp.tile([128, C], F16, tag="ob", name="ob")
                    if half == 0:
                        pos[j] = pC.tile([128, 512], F32, tag=f"po{j % 2}",
                                         name=f"po{j % 2}")
                    po = pos[j]
                    for h2 in range(2):
                        h = half * 2 + h2
                        nc.tensor.matmul(
                            po, lhsT=OT[:, h, g, :],
                            rhs=wo_sb[:, h, j * 512:(j + 1) * 512],
                            start=(h == 0), stop=(h == HPC - 1))
                    if half == 1:
                        ob = obs[g]
                        if j % 2 == 0:
                            nc.vector.tensor_copy(
                                ob[:, j * 512:(j + 1) * 512], po)
                        else:
                            nc.scalar.activation(
                                ob[:, j * 512:(j + 1) * 512], po, AF.Identity)
                        del pos[j]
                        if j == 3:
                            nc.sync.dma_start(out[g * 128:(g + 1) * 128, :],
                                              ob)
                            del obs[g]

                prev_g = None
                pending_tail = [None]

                def emit_tail():
                    if pending_tail[0] is None:
                        return
                    t_hp, t_i, t_OTp, t_rsB = pending_tail[0]
                    pending_tail[0] = None
                    for h2 in range(2):
                        rBsb = ptp.tile([128, 128], F16, tag=f"rBsb{h2}",
                                        name=f"rBsb{h2}")
                        with nc.allow_low_precision(
                                reason="softmax reciprocal f16"):
                            nc.vector.reciprocal(rBsb, t_rsB[h2])
                        nc.vector.tensor_mul(OT[:, t_hp * 2 + h2, t_i, :],
                                             t_OTp[h2], rBsb)

                for i in range(GPB):
                    cq = list(range(8)) if prev_g is not None else []
                    qoff = i * 128
                    nblk = i + 1
                    nq = (nblk + 3) // 4
                    for hp in range(HPAIRS):
                        # one PSUM bank holds OTp(h0|h1) and rsB(h0|h1)
                        combo = pO.tile([128, 512], F32, tag="combo",
                                        name="combo")
                        OTp = [combo[:, h2 * 128:(h2 + 1) * 128]
                               for h2 in range(2)]
                        rsB = [combo[:, 256 + h2 * 128:256 + (h2 + 1) * 128]
                               for h2 in range(2)]

                        def s_quartet(qt):
                            kb0 = qt * 4
                            nkb = min(4, nblk - kb0)
                            PTs = {}
                            for h2 in range(2):
                                hh = hp * 2 + h2
                                ST = pST.tile([128, 512], F32, tag=f"ST{h2}",
                                              name=f"ST{h2}")
                                for kk in range(nkb):
                                    kb = kb0 + kk
                                    nc.tensor.matmul(
                                        ST[:, kk * 128:(kk + 1) * 128],
                                        lhsT=kT[:, hh, kb * 128:
                                                (kb + 1) * 128],
                                        rhs=qT[:, hh, qoff:qoff + 128],
                                        start=True, stop=True)
                                PT = ptp.tile([128, 512], F16, tag=f"PT{h2}",
                                              name=f"PT{h2}")
                                nc.scalar.activation(
                                    PT[:, :nkb * 128], ST[:, :nkb * 128],
                                    AF.Exp, scale=SCL)
                                if kb0 + nkb - 1 == i:  # diagonal block
                                    kkd = i - kb0
                                    nc.vector.tensor_mul(
                                        PT[:, kkd * 128:(kkd + 1) * 128],
                                        PT[:, kkd * 128:(kkd + 1) * 128],
                                        tri01)
                                PTs[h2] = PT
                            return PTs

                        def pv_quartet(qt, PTs):
                            kb0 = qt * 4
                            nkb = min(4, nblk - kb0)
                            for h2 in range(2):
                                hh = hp * 2 + h2
                                for kk in range(nkb):
                                    kb = kb0 + kk
                                    nc.tensor.matmul(
                                        OTp[h2], lhsT=vN[:, hh, kb, :],
                                        rhs=PTs[h2][:, kk * 128:
                                                    (kk + 1) * 128],
                                        start=(h2 == 0 and kb == 0),
                                        stop=False, skip_group_check=True)
                                for kk in range(nkb):
                                    kb = kb0 + kk
                                    nc.tensor.matmul(
                                        rsB[h2], lhsT=onesmat,
                                        rhs=PTs[h2][:, kk * 128:
                                                    (kk + 1) * 128],
                                        start=False,
                                        stop=(h2 == 1 and kb == i),
                                        skip_group_check=True)

                        # software pipeline: S/exp one quartet ahead of PV;
                        # phase C chunks of the previous row fill the
                        # first-quartet exp latency and inter-quartet gaps.
                        # The normalization tail of the PREVIOUS pass is
                        # emitted right after this pass's first S quartet so
                        # its DVE ops queue behind the mask-add/exp chain
                        # instead of stalling it.
                        pending = s_quartet(0)
                        emit_tail()
                        if cq:
                            phase_c_chunk(prev_g, cq.pop(0))
                        if cq:
                            phase_c_chunk(prev_g, cq.pop(0))
                        for qt in range(nq):
                            nxt = s_quartet(qt + 1) if qt + 1 < nq else None
                            pv_quartet(qt, pending)
                            if cq:
                                phase_c_chunk(prev_g, cq.pop(0))
                            pending = nxt
                        pending_tail[0] = (hp, i, OTp, rsB)
                    while cq:
                        phase_c_chunk(prev_g, cq.pop(0))
                    prev_g = i
                emit_tail()
                for ch in range(8):
                    phase_c_chunk(prev_g, ch)

        if reps == 1:
            body()
        else:
            with tc.For_i(0, reps, 1):
                body()


def build_nc(reps: int = 1):
    nc = bacc.Bacc("TRN2", target_bir_lowering=False, debug=False)
    with tile.TileContext(nc) as tc:
        _emit(tc, reps)
    nc.compile()
    return nc


def make_in_maps(x, Wq, bq, Wk, bk, Wv, bv, Wo, bo):
    xb = {}
    for b_i in range(B):
        xb[b_i] = np.ascontiguousarray(
            np.asarray(x[b_i], dtype=np.float32).T).astype(np.float16)
    in_maps = []
    for cid in range(NCORES):
        b_i, hg = divmod(cid, NCORES // B)
        cols = slice(hg * HD, (hg + 1) * HD)
        bq_c = np.asarray(bq[cols], dtype=np.float32)
        bk_c = np.asarray(bk[cols], dtype=np.float32)
        bqk_c = np.stack(
            [bq_c[h * 128:(h + 1) * 128] for h in range(HPC)]
            + [bk_c[h * 128:(h + 1) * 128] for h in range(HPC)], axis=1)
        in_maps.append({
            "xT": xb[b_i],
            "wq": np.ascontiguousarray(Wq[:, cols]).astype(np.float16),
            "wk": np.ascontiguousarray(Wk[:, cols]).astype(np.float16),
            "wv": np.ascontiguousarray(Wv[:, cols]).astype(np.float16),
            "wo": np.ascontiguousarray(Wo[cols, :]).astype(np.float16),
            "bqk": np.ascontiguousarray(bqk_c),
            "bv2": np.asarray(bv[cols], dtype=np.float16)[None, :],
        })
    return in_maps


def gather(results, bo):
    acc = np.zeros((B, T, C), dtype=np.float32)
    for cid, r in enumerate(results):
        acc[cid // (NCORES // B)] += r["out"].astype(np.float32)
    acc += np.asarray(bo, dtype=np.float32)[None, None, :]
    return acc


_NC_CACHE = {}


def kernel(x, Wq, bq, Wk, bk, Wv, bv, Wo, bo, train=None, **_unused):
    if "nc" not in _NC_CACHE:
        _NC_CACHE["nc"] = build_nc(reps=1)
    nc = _NC_CACHE["nc"]
    in_maps = make_in_maps(x, Wq, bq, Wk, bk, Wv, bv, Wo, bo)
    res = bass_utils.run_bass_kernel_spmd(nc, in_maps, core_ids=list(range(NCORES)))
    return gather(res.results, bo).astype(np.float32)
